# revision 10
# baseline (speedup 1.0000x reference)
"""Causal MQA self-attention (RoPE + RMS-norm on q/k) on 8 TRN2 NeuronCores.

Sharding: core c -> (batch b = c//4, head-group g = c%4 of 4 heads).
Each core computes, for its batch and its 4 heads:
  q/k/v projections -> RoPE -> RMS-norm -> causal attention -> partial
  output projection out_part = attn_out_g @ wo[:, g].T  ([S, HID] bf16).
Host sums the 4 per-group partials of each batch in f32.

v3 structure:
- Head-packed attention, processed per 128-q tile with a one-tile lag:
  scores for (qt, kt) are ONE 512-wide matmul over all 4 heads
  (rhs = qT_all[:, :, qt]), exp'd in ONE 512-wide ACT op whose
  per-partition scale folds in 1/(rms_k*sqrt(D)) == 1/sqrt(sum k_rope^2).
  Causality is exact at 128-tile granularity; the exp load spreads evenly
  across the whole timeline (each q-tile's attention runs right after its
  projection tile), so there is no end-of-sequence exp burst.
- RoPE in bf16 on DVE (2x mode, no ACT copies); RMS scale via
  exp(-0.5*ln(ms+eps)) keeps every ACT function in one activation table.
- All DRAM inputs host-retiled to 4KB+ contiguous runs per partition;
  load order lets the first q-proj matmul start a few us in.
- probs @ [v|ones] per (qt, h) gives the softmax denominator on the
  q-partition for free; attention output transposes on the PE feeding
  the per-row output projection (emitted with a one-tile lag as well).
"""

import ml_dtypes
import numpy as np

import concourse.bass as bass
import concourse.mybir as mybir
import concourse.tile as tile
from concourse import bacc
from concourse.bass_utils import run_bass_kernel_spmd
from concourse.masks import make_identity

# problem dims (hardcoded per contract)
B, S, HID, H, D = 2, 2048, 2048, 16, 128
NCORES = 8
GROUPS = 4              # head-groups = cores per batch
HG = H // GROUPS        # heads per core
DG = HG * D             # 512 projected q dims per core
NT = S // 128           # 16 sequence tiles
HT = HID // 128         # 16 hidden tiles
EPS = 1.1920928955078125e-07
USE_LN = True           # rsqrt via exp(-0.5 ln x) on ACT (single act table)

f32 = mybir.dt.float32
bf16 = mybir.dt.bfloat16

TRACE = False           # test harness may flip this for NTFF profiling
LAST = {}               # last BassKernelResults, for the test harness

_compiled = None


def _emit(nc, xR, wqR, wkvR, woR, csR, snR, cmw4, out):
    add = mybir.AluOpType.add
    Exp = mybir.ActivationFunctionType.Exp
    Ln = mybir.ActivationFunctionType.Ln
    Sqrt = mybir.ActivationFunctionType.Sqrt

    with tile.TileContext(nc) as tc:
        with (
            tc.tile_pool(name="consts", bufs=1) as consts,
            tc.tile_pool(name="bigp", bufs=1) as bigp,
            tc.tile_pool(name="xsp", bufs=3) as xsp,
            tc.tile_pool(name="rsp", bufs=2) as rsp,
            tc.tile_pool(name="ksp", bufs=2) as ksp,
            tc.tile_pool(name="smp", bufs=2) as smp,
            tc.tile_pool(name="qnp", bufs=3) as qnp,
            tc.tile_pool(name="ptp", bufs=2) as ptp,
            tc.tile_pool(name="otp", bufs=3) as otp,
            tc.tile_pool(name="ocp", bufs=4) as ocp,
            tc.tile_pool(name="psA", bufs=1, space="PSUM") as psA,
            tc.tile_pool(name="psS", bufs=3, space="PSUM") as psS,
            tc.tile_pool(name="psX", bufs=2, space="PSUM") as psX,
            tc.tile_pool(name="psW", bufs=2, space="PSUM") as psW,
        ):
            # ---- constants (no DMA deps) ----
            ident = consts.tile([128, 128], bf16)
            make_identity(nc, ident)

            xs_tiles = {}

            def prefetch_xs(st):
                if st >= NT or st in xs_tiles:
                    return
                xs = xsp.tile([128, HT, 128], bf16, tag="xs")
                nc.sync.dma_start(xs, xR[:, st, :, :])
                xs_tiles[st] = xs

            # priority DMA order: x tile 0, wq chunks, cos/sin, wkv
            prefetch_xs(0)
            wq_c = []
            for c in range(4):
                w = bigp.tile([128, 4, DG], bf16, tag=f"wq{c}")
                nc.sync.dma_start(w, wqR[:, c, :, :])
                wq_c.append(w)
            cs4a = bigp.tile([128, 2, DG], bf16, tag="cs4a")
            nc.sync.dma_start(cs4a, csR[:, 0:2, :])
            sn4a = bigp.tile([128, 2, DG], bf16, tag="sn4a")
            nc.sync.dma_start(sn4a, snR[:, 0:2, :])
            wkv_sb = bigp.tile([128, HT, 256], bf16, tag="wkv")
            nc.sync.dma_start(wkv_sb, wkvR)
            prefetch_xs(1)
            cs4b = bigp.tile([128, NT - 2, DG], bf16, tag="cs4b")
            nc.sync.dma_start(cs4b, csR[:, 2:NT, :])
            sn4b = bigp.tile([128, NT - 2, DG], bf16, tag="sn4b")
            nc.sync.dma_start(sn4b, snR[:, 2:NT, :])

            def cs_at(st):
                if st < 2:
                    return cs4a[:, st, :], sn4a[:, st, :]
                return cs4b[:, st - 2, :], sn4b[:, st - 2, :]
            cmw_sb = consts.tile([128, DG], bf16)   # tri mask x4 heads
            nc.sync.dma_start(cmw_sb, cmw4)

            qT_all = bigp.tile([128, HG, S], bf16, tag="qT")   # [d, h, s]
            kT_sb = bigp.tile([128, S], bf16, tag="kT")        # [d, s]
            vvb = bigp.tile([128, NT, 132], bf16, tag="vv")    # [k%128, k//128, d|1]
            nc.vector.memset(vvb[:, :, 128:132], 1.0)
            srtk_all = bigp.tile([128, NT], f32, tag="srtk")   # 1/sqrt(sum k^2)

            wo_holder = []

            def emit_wo_dma():
                wo_sb = bigp.tile([128, HG, HID], bf16, tag="wo")
                nc.sync.dma_start(wo_sb, woR)
                wo_holder.append(wo_sb)

            def rsqrt_newton(y, m, tmp_pool):
                """y = 1/sqrt(m) elementwise on DVE (no ACT table funcs).

                3 Newton steps from the linear seed y0 = 1.5 - m/2;
                converges to ~2e-4 for m in [0.25, 1.3] (rms of rope'd
                rows is ~sqrt(2/3), so m stays well inside).
                """
                mult = mybir.AluOpType.mult
                add = mybir.AluOpType.add
                w = list(m.shape)
                nc.vector.tensor_scalar(y, m, -0.5, 1.5, mult, add)
                for _ in range(3):
                    y2 = tmp_pool.tile(w, f32, tag="nw1")
                    nc.vector.tensor_mul(y2, y, y)
                    nc.vector.tensor_mul(y2, y2, m)
                    nc.vector.tensor_scalar(y2, y2, -0.5, 1.5, mult, add)
                    nc.vector.tensor_mul(y, y, y2)

            def emit_st(st):
                xs = xs_tiles.pop(st)
                prefetch_xs(st + 2)

                qp = psA.tile([128, DG], f32, tag="qp")
                for t in range(HT):
                    nc.tensor.matmul(
                        qp, lhsT=xs[:, t, :], rhs=wq_c[t // 4][:, t % 4, :],
                        start=(t == 0), stop=(t == HT - 1))
                kvp = psW.tile([128, DG], f32, tag="w")
                for t in range(HT):
                    nc.tensor.matmul(
                        kvp[:, 0:256], lhsT=xs[:, t, :], rhs=wkv_sb[:, t, :],
                        start=(t == 0), stop=(t == HT - 1))

                # ---- q: psum -> bf16, RoPE, RMS, transpose ----
                qs = rsp.tile([128, HG, 128], bf16, tag="qs")
                nc.scalar.copy(qs, qp.rearrange("p (h d) -> p h d", h=HG))
                cs4, sn4 = cs_at(st)
                cst = cs4[0:128, 0:128]
                snt = sn4[0:128, 0:128]
                m1 = rsp.tile([128, HG, 128], bf16, tag="m1")
                m2 = rsp.tile([128, HG, 128], bf16, tag="m2")
                nc.vector.tensor_mul(
                    m1, qs, cs4.rearrange("p (h d) -> p h d", h=HG))
                nc.vector.tensor_mul(
                    m2, qs, sn4.rearrange("p (h d) -> p h d", h=HG))
                t1 = rsp.tile([128, HG, 128], bf16, tag="t1")
                nc.vector.tensor_add(t1[:, :, 0:64], m1[:, :, 0:64],
                                     m2[:, :, 64:128])
                nc.vector.tensor_sub(t1[:, :, 64:128], m1[:, :, 64:128],
                                     m2[:, :, 0:64])
                # ---- k/v from kv psum (rope k before the rms reduces so
                # q and k share one Newton rsqrt on a [128, 5] tile) ----
                nc.vector.tensor_copy(vvb[:, st, 0:128], kvp[:, 128:256])
                ks = ksp.tile([128, 128], bf16, tag="ks")
                nc.scalar.copy(ks, kvp[:, 0:128])
                km1 = ksp.tile([128, 128], bf16, tag="km1")
                nc.vector.tensor_mul(km1, ks, cst)
                km2 = ksp.tile([128, 128], bf16, tag="km2")
                nc.vector.tensor_mul(km2, ks, snt)
                kt1 = ksp.tile([128, 128], bf16, tag="kt1")
                nc.vector.tensor_add(kt1[:, 0:64], km1[:, 0:64], km2[:, 64:128])
                nc.vector.tensor_sub(kt1[:, 64:128], km1[:, 64:128],
                                     km2[:, 0:64])

                sq = rsp.tile([128, HG, 128], f32, tag="sq")
                nc.vector.tensor_mul(sq, t1, t1)
                ksq = ksp.tile([128, 128], f32, tag="ksq")
                nc.vector.tensor_mul(ksq, kt1, kt1)
                ms5 = smp.tile([128, 5], f32, tag="ms5")
                nc.vector.tensor_reduce(ms5[:, 0:4], sq,
                                        axis=mybir.AxisListType.X, op=add)
                nc.vector.tensor_reduce(ms5[:, 4:5], ksq,
                                        axis=mybir.AxisListType.X, op=add)
                nc.vector.tensor_scalar_mul(ms5, ms5, 1.0 / D)
                si5 = smp.tile([128, 5], f32, tag="si5")
                rsqrt_newton(si5, ms5, smp)
                # fold the 1/sqrt(D) of 1/sqrt(sum_k) into the k scale
                nc.vector.tensor_scalar_mul(
                    srtk_all[:, st:st + 1], si5[:, 4:5], float(1.0 / np.sqrt(D)))

                for h in range(HG):
                    qn = qnp.tile([128, 128], bf16, tag="qn")
                    nc.vector.tensor_scalar_mul(qn, t1[:, h, :], si5[:, h:h + 1])
                    tp = psX.tile([128, DG], f32, tag="x")
                    nc.tensor.transpose(tp[:, 0:64].bitcast(bf16), qn, ident)
                    nc.scalar.copy(
                        qT_all[:, h, st * 128:(st + 1) * 128],
                        tp[:, 0:64].bitcast(bf16))
                tpk = psX.tile([128, DG], f32, tag="x")
                nc.tensor.transpose(tpk[:, 0:64].bitcast(bf16), kt1, ident)
                nc.scalar.copy(
                    kT_sb[:, st * 128:(st + 1) * 128],
                    tpk[:, 0:64].bitcast(bf16))

            def emit_scores(qt, pts, kts):
                """head-packed scores+exp for q-tile qt over k-tiles kts."""
                qrhs = qT_all[:, :, qt * 128:(qt + 1) * 128]
                for kt in kts:
                    sp = psS.tile([128, HG, 128], f32, tag="sp")
                    nc.tensor.matmul(sp, lhsT=kT_sb[:, kt * 128:(kt + 1) * 128],
                                     rhs=qrhs, start=True, stop=True)
                    nc.scalar.activation(
                        out=pts[:, kt, :].rearrange("p (h q) -> p h q", h=HG),
                        in_=sp, func=Exp, scale=srtk_all[:, kt:kt + 1])

            def emit_mask(qt, pts):
                nc.vector.tensor_mul(pts[:, qt, :], pts[:, qt, :], cmw_sb)

            def emit_pv(qt, h, pts, otile):
                op = psX.tile([128, DG], f32, tag="x")
                for kt in range(qt + 1):
                    nc.tensor.matmul(
                        op[:, 0:129],
                        lhsT=pts[:, kt, h * 128:(h + 1) * 128],
                        rhs=vvb[:, kt, 0:129],
                        start=(kt == 0), stop=(kt == qt))
                rc = smp.tile([128, 1], f32, tag="rc")
                nc.vector.reciprocal(rc, op[:, 128:129])
                on = qnp.tile([128, 128], bf16, tag="on")
                nc.vector.tensor_scalar_mul(on, op[:, 0:128], rc)
                tp = psX.tile([128, DG], f32, tag="x")
                nc.tensor.transpose(tp[:, 0:64].bitcast(bf16), on, ident)
                nc.vector.tensor_copy(
                    otile[:, h, :], tp[:, 0:64].bitcast(bf16))

            def emit_outproj(qt, otile):
                wo_sb = wo_holder[0]
                srow = qt * 128
                for cc in range(4):
                    wop = psW.tile([128, DG], f32, tag="w")
                    for h2 in range(HG):
                        nc.tensor.matmul(
                            wop, lhsT=otile[:, h2, :],
                            rhs=wo_sb[:, h2, cc * DG:(cc + 1) * DG],
                            start=(h2 == 0), stop=(h2 == HG - 1))
                    oc = ocp.tile([128, DG], bf16, tag="oc")
                    if cc % 2 == 0:
                        nc.vector.tensor_copy(oc, wop)
                    else:
                        nc.scalar.copy(oc, wop)
                    # out-writes go on the (otherwise idle) Pool DGE so their
                    # waits never head-of-line-block the SP input prefetches
                    nc.gpsimd.dma_start(
                        out[srow:srow + 128, cc * DG:(cc + 1) * DG], oc)

            # ================= schedule =================
            # Two-iteration software pipeline: iteration st projects tile
            # st (whose qn/kT chain runs on ACT/DVE through the whole
            # iteration), computes scores+exp for q-tile st-1, and runs
            # PV+outproj for q-tile st-2 -- PV/outproj pieces lead the PE
            # stream so the PE never waits on the fresh chain.
            sc_state = {}    # qt -> (pts, otile)

            def emit_iter(score_qt, bc_qt):
                pieces = []
                if bc_qt is not None:
                    bpts, bot = sc_state[bc_qt]
                    pieces = [("pv", h) for h in range(HG)] + [("out",)]
                kts = list(range(score_qt + 1)) if score_qt is not None else []
                if score_qt is not None:
                    pts = ptp.tile([128, NT, DG], bf16, tag="pt")
                    otile = otp.tile([128, HG, 128], bf16, tag="ot")
                    sc_state[score_qt] = (pts, otile)
                gi = 0
                while kts or gi < len(pieces):
                    if gi < len(pieces):
                        p = pieces[gi]
                        gi += 1
                        if p[0] == "pv":
                            emit_pv(bc_qt, p[1], bpts, bot)
                        else:
                            emit_outproj(bc_qt, bot)
                            del sc_state[bc_qt]
                    if kts:
                        grp, kts = kts[:3], kts[3:]
                        emit_scores(score_qt, pts, grp)
                        if not kts:
                            emit_mask(score_qt, pts)

            for st in range(NT):
                emit_iter(st - 1 if st >= 1 else None,
                          st - 2 if st >= 2 else None)
                emit_st(st)
                if st == 1:
                    emit_wo_dma()
            emit_iter(NT - 1, NT - 2)
            emit_iter(None, NT - 1)


def _build():
    nc = bacc.Bacc("TRN2", target_bir_lowering=False, debug=False,
                   num_devices=NCORES)
    xR = nc.dram_tensor("xR", [128, NT, HT, 128], bf16, kind="ExternalInput").ap()
    wqR = nc.dram_tensor("wqR", [128, 4, 4, DG], bf16, kind="ExternalInput").ap()
    wkvR = nc.dram_tensor("wkvR", [128, HT, 256], bf16, kind="ExternalInput").ap()
    woR = nc.dram_tensor("woR", [128, HG, HID], bf16, kind="ExternalInput").ap()
    csR = nc.dram_tensor("csR", [128, NT, DG], bf16, kind="ExternalInput").ap()
    snR = nc.dram_tensor("snR", [128, NT, DG], bf16, kind="ExternalInput").ap()
    cmw4 = nc.dram_tensor("cmw4", [128, DG], bf16, kind="ExternalInput").ap()
    out = nc.dram_tensor("out", [S, HID], bf16, kind="ExternalOutput").ap()
    _emit(nc, xR, wqR, wkvR, woR, csR, snR, cmw4, out)
    nc.compile()
    return nc


def _get_compiled():
    global _compiled
    if _compiled is None:
        _compiled = _build()
    return _compiled


def kernel(x, cos, sin, wq, wk, wv, wo):
    nc = _get_compiled()
    x = np.asarray(x, np.float32)
    cos = np.asarray(cos, np.float32)
    sin = np.asarray(sin, np.float32)
    wq = np.asarray(wq, np.float32)
    wk = np.asarray(wk, np.float32)
    wv = np.asarray(wv, np.float32)
    wo = np.asarray(wo, np.float32)

    bf = ml_dtypes.bfloat16

    def part_tile(a2d, inner):
        t = a2d.shape[0] // 128
        return np.ascontiguousarray(
            a2d.reshape(t, 128, inner).transpose(1, 0, 2).astype(bf))

    xRs = []
    for b in range(B):
        xT = x[b].T  # [HID, S]
        xr = xT.reshape(HT, 128, NT, 128).transpose(1, 2, 0, 3)
        xRs.append(np.ascontiguousarray(xr.astype(bf)))
    wqRs = []
    for g in range(GROUPS):
        wqT = wq[g * DG:(g + 1) * DG].T  # [HID, DG]
        wqr = wqT.reshape(4, 4, 128, DG).transpose(2, 0, 1, 3)
        wqRs.append(np.ascontiguousarray(wqr.astype(bf)))
    wkvR = part_tile(np.concatenate([wk, wv], 0).T, 256)
    woRs = [part_tile(wo[:, g * DG:(g + 1) * DG].T, HID)
            for g in range(GROUPS)]
    csR = part_tile(np.tile(np.concatenate([cos, cos], 1), (1, HG)), DG)
    snR = part_tile(np.tile(np.concatenate([sin, sin], 1), (1, HG)), DG)
    tri = np.triu(np.ones((128, 128), np.float32))   # 1 where k <= q
    cmw4 = np.ascontiguousarray(
        np.tile(tri, (1, HG)).astype(bf))

    in_maps = []
    for c in range(NCORES):
        b, g = divmod(c, GROUPS)
        in_maps.append({
            "xR": xRs[b], "wqR": wqRs[g], "wkvR": wkvR, "woR": woRs[g],
            "csR": csR, "snR": snR, "cmw4": cmw4,
        })
    res = run_bass_kernel_spmd(nc, in_maps, list(range(NCORES)), trace=TRACE)
    LAST["res"] = res
    outs = [r["out"] for r in res.results]
    final = np.empty((B, S, HID), np.float32)
    for b in range(B):
        final[b] = (outs[GROUPS * b].astype(np.float32)
                    + outs[GROUPS * b + 1].astype(np.float32)
                    + outs[GROUPS * b + 2].astype(np.float32)
                    + outs[GROUPS * b + 3].astype(np.float32))
    return final
